# revision 13
# baseline (speedup 1.0000x reference)
"""Bass/Tile TRN2 kernel for GroupedQueryAttention (B=2, T=2048, D=2048,
32 Q heads / 8 KV heads, hd=64, RoPE, causal), sharded over 8 NeuronCores
by KV head (1 KV head + 4 Q heads per core; wo row-sharded, partials
summed on host). bf16 datapath, fp32 PSUM accumulation.

Schedule: projection work for future chunks and the attention i-loop are
interleaved at emission time so the PE never starves behind the ACT
engine's exp stream (which would trigger HAM downclocking); the output
projection is deferred to a dense tail phase."""

import sys

for _p in ("/opt/trn_rl_repo",):
    if _p not in sys.path:
        sys.path.insert(0, _p)

import numpy as np
import ml_dtypes

import concourse.bass as bass
import concourse.mybir as mybir
import concourse.tile as tile
from concourse import bacc
from concourse.bass_utils import run_bass_kernel_spmd

F32 = mybir.dt.float32
BF16 = mybir.dt.bfloat16
NPBF16 = ml_dtypes.bfloat16
P = 128
HD = 64          # head dim
NHL = 4          # q heads per core
CH = 512         # token chunk (matmul free dim)
NCORES = 8


def build_program(B=2, T=2048, D=2048):
    """Emit the per-core SPMD program. Identical on all cores; inputs differ."""
    BT = B * T
    KT = D // P            # contraction tiles for projections
    NCH = BT // CH         # 512-token chunks over all batches
    NJ = T // CH           # tq chunks per batch
    NI = T // P            # tk tiles per batch
    NTT = BT // P          # token tiles over all batches

    nc = bacc.Bacc(None, target_bir_lowering=False, debug=False)

    xT_d = nc.dram_tensor("xT", [D, BT], BF16, kind="ExternalInput")
    wq_d = nc.dram_tensor("wqT", [D, 256], BF16, kind="ExternalInput")
    wkv_d = nc.dram_tensor("wkvT", [D, 128], BF16, kind="ExternalInput")
    wo_d = nc.dram_tensor("woT", [256, D], BF16, kind="ExternalInput")
    cs_d = nc.dram_tensor("cs", [P, T], BF16, kind="ExternalInput")
    sn_d = nc.dram_tensor("sn", [P, T], BF16, kind="ExternalInput")
    perm_d = nc.dram_tensor("perm", [P, P], BF16, kind="ExternalInput")
    id64_d = nc.dram_tensor("id64", [HD, HD], F32, kind="ExternalInput")
    dmask_d = nc.dram_tensor("dmask", [P, P], F32, kind="ExternalInput")
    out_d = nc.dram_tensor("out", [BT, D], BF16, kind="ExternalOutput")

    with tile.TileContext(nc) as tc:
        with (
            tc.tile_pool(name="persist", bufs=1) as persist,
            tc.tile_pool(name="xk", bufs=6) as xkp,
            tc.tile_pool(name="rtmp", bufs=2) as rtmp,
            tc.tile_pool(name="pp", bufs=6) as pp,
            tc.tile_pool(name="att", bufs=NCH) as att,
            tc.tile_pool(name="otp", bufs=6) as otp,
        ):
            wq_sb = persist.tile([P, KT, 256], BF16, tag="wq")
            wkv_sb = persist.tile([P, KT, 128], BF16, tag="wkv")
            wo_sb = persist.tile([P, 2, D], BF16, tag="wo")
            cs_sb = persist.tile([P, T], BF16, tag="cs")
            sn_sb = persist.tile([P, T], BF16, tag="sn")
            perm_sb = persist.tile([P, P], BF16, tag="perm")
            id64_sb = persist.tile([HD, HD], F32, tag="id64")
            dmask_sb = persist.tile([P, 2, P], F32, tag="dmask")
            q_sb = persist.tile([P, 2, BT], BF16, tag="qcache")
            k_sb = persist.tile([P, B, T], BF16, tag="kcache")
            v_sb = persist.tile([P, NTT, HD + 1], BF16, tag="vcache")

            nc.sync.dma_start(wq_sb[:], wq_d[:].rearrange("(ko p) m -> p ko m", p=P))
            nc.sync.dma_start(wkv_sb[:], wkv_d[:].rearrange("(ko p) m -> p ko m", p=P))
            nc.sync.dma_start(wo_sb[:], wo_d[:].rearrange("(fo p) n -> p fo n", p=P))
            nc.sync.dma_start(cs_sb[:], cs_d[:])
            nc.sync.dma_start(sn_sb[:], sn_d[:])
            nc.sync.dma_start(perm_sb[:], perm_d[:])
            nc.sync.dma_start(id64_sb[:], id64_d[:])
            nc.sync.dma_start(dmask_sb[:, 0, :], dmask_d[:])
            nc.sync.dma_start(dmask_sb[:, 1, :], dmask_d[:])
            nc.vector.memset(v_sb[:, :, HD:HD + 1], 1.0)

            # ---- main interleaved phase: projections + RoPE + attention
            with (
                tc.tile_pool(name="pa", bufs=3, space="PSUM") as pa,
                tc.tile_pool(name="pm", bufs=1, space="PSUM") as pm,
                tc.tile_pool(name="po", bufs=2, space="PSUM") as po,
                tc.tile_pool(name="ps", bufs=1, space="PSUM") as ps,
            ):

                def proj_units(ch):
                    """Generator: projection+RoPE of one 512-token chunk,
                    yielded in small PE-sized units for interleaving."""
                    b = ch // NJ
                    tcol = ch * CH               # global token col
                    kcol = CH * (ch % NJ)        # within-batch token col
                    kvp = pa.tile([P, CH], F32, tag="pacc")
                    q0p = pa.tile([P, CH], F32, tag="pacc")
                    q1p = pa.tile([P, CH], F32, tag="pacc")
                    for k in range(KT):
                        xk = xkp.tile([P, CH], BF16, tag="xk")
                        nc.sync.dma_start(
                            xk[:], xT_d[k * P:(k + 1) * P, tcol:tcol + CH])
                        st = (k == 0)
                        sp = (k == KT - 1)
                        nc.tensor.matmul(kvp[:], wkv_sb[:, k, :], xk[:],
                                         start=st, stop=sp)
                        nc.tensor.matmul(q0p[:], wq_sb[:, k, 0:P], xk[:],
                                         start=st, stop=sp)
                        nc.tensor.matmul(q1p[:], wq_sb[:, k, P:256], xk[:],
                                         start=st, stop=sp)
                        if k % 2 == 1:
                            yield
                    csl = cs_sb[:, kcol:kcol + CH]
                    snl = sn_sb[:, kcol:kcol + CH]
                    # q RoPE: rope(q) = q*cos + (P.T@q)*sin
                    for hx, (ht, qp) in enumerate(((0, q0p), (1, q1p))):
                        qs = rtmp.tile([P, CH], BF16, tag="ropea")
                        if hx == 0:
                            nc.scalar.copy(qs[:], qp[:])
                        else:
                            nc.vector.tensor_copy(qs[:], qp[:])
                        qsw = pm.tile([P, CH], F32, tag="pswap")
                        nc.tensor.matmul(qsw[:], perm_sb[:], qs[:],
                                         start=True, stop=True)
                        dst = q_sb[:, ht, tcol:tcol + CH]
                        nc.gpsimd.tensor_mul(dst, qs[:], csl)
                        t2 = rtmp.tile([P, CH], BF16, tag="ropeb")
                        nc.vector.tensor_mul(t2[:], qsw[:], snl)
                        nc.vector.tensor_add(dst, dst, t2[:])
                        yield
                    # k RoPE (rows 0:64 of kv psum), then duplicate to 64:128
                    ks = rtmp.tile([HD, CH], BF16, tag="ropek")
                    nc.scalar.copy(ks[:], kvp[0:HD, :])
                    ksw_full = pm.tile([P, CH], F32, tag="pswap")
                    ksw = ksw_full[0:HD, :]
                    nc.tensor.matmul(ksw, perm_sb[0:HD, 0:HD], ks[:],
                                     start=True, stop=True)
                    kdst = k_sb[0:HD, b, kcol:kcol + CH]
                    nc.vector.tensor_mul(kdst, ks[:], cs_sb[0:HD, kcol:kcol + CH])
                    t2 = rtmp.tile([HD, CH], BF16, tag="ropekb")
                    nc.vector.tensor_mul(t2[:], ksw, sn_sb[0:HD, kcol:kcol + CH])
                    nc.vector.tensor_add(kdst, kdst, t2[:])
                    nc.gpsimd.tensor_copy(k_sb[HD:P, b, kcol:kcol + CH], kdst)
                    yield
                    # v: copy rows 64:128, transpose to token-major
                    vs = rtmp.tile([HD, CH], F32, tag="ropev")
                    nc.scalar.copy(vs[:], kvp[HD:P, :])
                    for tt in range(CH // P):
                        vtp = pm.tile([P, CH], F32, tag="pswap")
                        nc.tensor.transpose(vtp[:, 0:HD],
                                            vs[:, tt * P:(tt + 1) * P],
                                            id64_sb[:])
                        nc.vector.tensor_copy(
                            v_sb[:, ch * (CH // P) + tt, 0:HD], vtp[:, 0:HD])
                    yield

                projq = [proj_units(ch) for ch in range(NCH)]
                prog = [0]  # index of first non-exhausted generator

                def pull(n):
                    """Advance future-chunk projection emission by n units."""
                    while n > 0 and prog[0] < NCH:
                        try:
                            next(projq[prog[0]])
                            n -= 1
                        except StopIteration:
                            prog[0] += 1

                def exhaust(ch):
                    """Run projection generators up to chunk ch to completion."""
                    while prog[0] <= ch:
                        try:
                            next(projq[prog[0]])
                        except StopIteration:
                            prog[0] += 1

                outproj = []  # deferred out-proj units: (og_c, tcol)

                for b in range(B):
                    for j in range(NJ):
                        ch = b * NJ + j
                        tcol = ch * CH
                        exhaust(ch)  # q/k/v for this chunk must be ready
                        imax = (CH // P) * j + (CH // P) - 1
                        og_c = att.tile([P, 2, CH], BF16, tag="og",
                                        name=f"og_{ch}")
                        for hp in range(NHL // 2):
                            ot_acc = [po.tile([HD + 1, CH], F32, tag="po",
                                              name=f"po_{ch}_{h}")
                                      for h in (2 * hp, 2 * hp + 1)]
                            for i in range(imax + 1):
                                c0 = max(0, P * i - CH * j)
                                sp2 = ps.tile([P, 2, CH], F32, tag="ps")
                                for hi, h in enumerate((2 * hp, 2 * hp + 1)):
                                    hb = HD * (h % 2)
                                    ht = h // 2
                                    nc.tensor.matmul(
                                        sp2[:, hi, c0:CH],
                                        k_sb[hb:hb + HD, b, P * i:P * (i + 1)],
                                        q_sb[hb:hb + HD, ht, tcol + c0:tcol + CH],
                                        start=True, stop=True)
                                if P * i >= CH * j:  # diagonal: causal mask
                                    nc.vector.tensor_add(
                                        sp2[:, :, c0:c0 + P],
                                        sp2[:, :, c0:c0 + P],
                                        dmask_sb[:])
                                pt2 = pp.tile([P, 2, CH], BF16, tag="pt")
                                nc.scalar.activation(
                                    pt2[:, :, c0:CH], sp2[:, :, c0:CH],
                                    mybir.ActivationFunctionType.Exp,
                                    scale=0.125)
                                for hi in range(2):
                                    nc.tensor.matmul(
                                        ot_acc[hi][:, c0:CH],
                                        v_sb[:, b * NI + i, :],
                                        pt2[:, hi, c0:CH],
                                        start=(i == 0), stop=(i == imax),
                                        skip_group_check=True)
                                pull(3)
                            for hi, h in enumerate((2 * hp, 2 * hp + 1)):
                                hb = HD * (h % 2)
                                ht = h // 2
                                # softmax denominator row -> SBUF partition 0
                                # (recip_approx misreads PSUM at partition
                                # offset 64), approx-reciprocal, broadcast
                                # across partitions on GpSimd
                                otr = otp.tile([1, CH], F32, tag="otr")
                                nc.vector.tensor_copy(
                                    otr[:], ot_acc[hi][HD:HD + 1, :])
                                dr = otp.tile([1, CH], F32, tag="dr")
                                nc.vector.reciprocal_approx_fast(
                                    dr[:], otr[:])
                                lr = otp.tile([HD, CH], F32, tag="lr")
                                nc.gpsimd.partition_broadcast(lr[:], dr[:])
                                nc.vector.tensor_mul(
                                    og_c[hb:hb + HD, ht, :],
                                    ot_acc[hi][0:HD, :], lr[:])
                            pull(2)
                        outproj.append((og_c, tcol))

            # ---- dense out-projection tail
            with tc.tile_pool(name="pout", bufs=4, space="PSUM") as pout:
                for u, (og_c, tcol) in enumerate(outproj):
                    for tt in range(CH // P):
                        for dc in range(D // CH):
                            op = pout.tile([P, CH], F32, tag="pout")
                            for ft in range(2):
                                nc.tensor.matmul(
                                    op[:],
                                    og_c[:, ft, tt * P:(tt + 1) * P],
                                    wo_sb[:, ft, dc * CH:(dc + 1) * CH],
                                    start=(ft == 0), stop=(ft == 1))
                            ob = otp.tile([P, CH], BF16, tag="ob")
                            if (tt * (D // CH) + dc) % 2 == 0:
                                nc.vector.tensor_copy(ob[:], op[:])
                            else:
                                nc.scalar.copy(ob[:], op[:])
                            nc.sync.dma_start(
                                out_d[tcol + tt * P:tcol + (tt + 1) * P,
                                      dc * CH:(dc + 1) * CH],
                                ob[:])
    nc.compile()
    return nc


def host_prep(x, wq, wk, wv, wo, cos, sin, core, B=2, T=2048, D=2048):
    """Per-core input map. Core c owns KV head c and Q heads 4c..4c+3."""
    BT = B * T
    xT = np.ascontiguousarray(x.reshape(BT, D).T.astype(NPBF16))
    wqT = np.ascontiguousarray(wq[256 * core:256 * (core + 1)].T.astype(NPBF16))
    wkvT = np.ascontiguousarray(
        np.concatenate([wk[HD * core:HD * (core + 1)],
                        wv[HD * core:HD * (core + 1)]], axis=0).T.astype(NPBF16))
    woT = np.ascontiguousarray(wo[:, 256 * core:256 * (core + 1)].T.astype(NPBF16))
    idx = (np.arange(P) % HD) // 2
    cs = np.ascontiguousarray(cos[:T, idx].T.astype(NPBF16))
    sn = np.ascontiguousarray(sin[:T, idx].T.astype(NPBF16))
    perm = np.zeros((P, P), dtype=NPBF16)
    ii = np.arange(0, P, 2)
    perm[ii, ii + 1] = 1.0
    perm[ii + 1, ii] = -1.0
    dmask = np.where(np.arange(P)[:, None] <= np.arange(P)[None, :],
                     0.0, -1e10).astype(np.float32)
    return {
        "xT": xT, "wqT": wqT, "wkvT": wkvT, "woT": woT,
        "cs": cs, "sn": sn, "perm": perm,
        "id64": np.eye(HD, dtype=np.float32), "dmask": dmask,
    }


_CACHE = {}


def _get_program(B, T, D):
    key = (B, T, D)
    if key not in _CACHE:
        _CACHE[key] = build_program(B, T, D)
    return _CACHE[key]


def run_on_hw(x, wq, wk, wv, wo, cos, sin, B=2, T=2048, D=2048, trace=False, **kw):
    nc = _get_program(B, T, D)
    in_maps = [host_prep(x, wq, wk, wv, wo, cos, sin, c, B, T, D)
               for c in range(NCORES)]
    res = run_bass_kernel_spmd(nc, in_maps, list(range(NCORES)), trace=trace, **kw)
    parts = [np.asarray(r["out"], dtype=np.float32) for r in res.results]
    out = sum(parts).astype(np.float32).reshape(B, T, D)
    return out, res


def kernel(x, mask, wq, wk, wv, wo, cos, sin):
    x = np.asarray(x, dtype=np.float32)
    out, _ = run_on_hw(np.asarray(x, np.float32), np.asarray(wq, np.float32),
                       np.asarray(wk, np.float32), np.asarray(wv, np.float32),
                       np.asarray(wo, np.float32), np.asarray(cos, np.float32),
                       np.asarray(sin, np.float32))
    return out


# revision 14
# speedup vs baseline: 1.3474x; 1.3474x over previous
"""Bass/Tile TRN2 kernel for GroupedQueryAttention (B=2, T=2048, D=2048,
32 Q heads / 8 KV heads, hd=64, RoPE, causal), sharded over 8 NeuronCores
by KV head (1 KV head + 4 Q heads per core; wo row-sharded, partials
summed on host). bf16 datapath (fp32 PSUM accumulation)."""

import sys

for _p in ("/opt/trn_rl_repo",):
    if _p not in sys.path:
        sys.path.insert(0, _p)

import numpy as np
import ml_dtypes

import concourse.bass as bass
import concourse.mybir as mybir
import concourse.tile as tile
from concourse import bacc
from concourse.bass_utils import run_bass_kernel_spmd

F32 = mybir.dt.float32
BF16 = mybir.dt.bfloat16
NPBF16 = ml_dtypes.bfloat16
P = 128
HD = 64          # head dim
NHL = 4          # q heads per core
CH = 512         # token chunk (matmul free dim)
NCORES = 8


def build_program(B=2, T=2048, D=2048):
    """Emit the per-core SPMD program. Identical on all cores; inputs differ."""
    BT = B * T
    KT = D // P            # contraction tiles for projections
    NCH = BT // CH         # 512-token chunks over all batches
    NJ = T // CH           # tq chunks per batch
    NI = T // P            # tk tiles per batch
    NTT = BT // P          # token tiles over all batches

    nc = bacc.Bacc(None, target_bir_lowering=False, debug=False)

    xT_d = nc.dram_tensor("xT", [D, BT], BF16, kind="ExternalInput")
    wq_d = nc.dram_tensor("wqT", [D, 256], BF16, kind="ExternalInput")
    wkv_d = nc.dram_tensor("wkvT", [D, 128], BF16, kind="ExternalInput")
    wo_d = nc.dram_tensor("woT", [256, D], BF16, kind="ExternalInput")
    cs_d = nc.dram_tensor("cs", [P, T], BF16, kind="ExternalInput")
    sn_d = nc.dram_tensor("sn", [P, T], BF16, kind="ExternalInput")
    perm_d = nc.dram_tensor("perm", [P, P], BF16, kind="ExternalInput")
    id64_d = nc.dram_tensor("id64", [HD, HD], F32, kind="ExternalInput")
    dmask_d = nc.dram_tensor("dmask", [P, P], F32, kind="ExternalInput")
    out_d = nc.dram_tensor("out", [BT, D], BF16, kind="ExternalOutput")

    with tile.TileContext(nc) as tc:
        with tc.tile_pool(name="persist", bufs=1) as persist:
            wq_sb = persist.tile([P, KT, 256], BF16, tag="wq")
            wkv_sb = persist.tile([P, KT, 128], BF16, tag="wkv")
            wo_sb = persist.tile([P, 2, D], BF16, tag="wo")
            cs_sb = persist.tile([P, T], BF16, tag="cs")
            sn_sb = persist.tile([P, T], BF16, tag="sn")
            perm_sb = persist.tile([P, P], BF16, tag="perm")
            id64_sb = persist.tile([HD, HD], F32, tag="id64")
            dmask_sb = persist.tile([P, 2, P], F32, tag="dmask")
            ones_sb = persist.tile([P, HD], BF16, tag="ones")
            q_sb = persist.tile([P, 2, BT], BF16, tag="qcache")
            k_sb = persist.tile([P, B, T], BF16, tag="kcache")
            v_sb = persist.tile([P, NTT, HD + 1], BF16, tag="vcache")

            nc.sync.dma_start(wq_sb[:], wq_d[:].rearrange("(ko p) m -> p ko m", p=P))
            nc.sync.dma_start(wkv_sb[:], wkv_d[:].rearrange("(ko p) m -> p ko m", p=P))
            nc.sync.dma_start(wo_sb[:], wo_d[:].rearrange("(fo p) n -> p fo n", p=P))
            nc.sync.dma_start(cs_sb[:], cs_d[:])
            nc.sync.dma_start(sn_sb[:], sn_d[:])
            nc.sync.dma_start(perm_sb[:], perm_d[:])
            nc.sync.dma_start(id64_sb[:], id64_d[:])
            nc.sync.dma_start(dmask_sb[:, 0, :], dmask_d[:])
            nc.sync.dma_start(dmask_sb[:, 1, :], dmask_d[:])
            nc.vector.memset(v_sb[:, :, HD:HD + 1], 1.0)
            nc.vector.memset(ones_sb[:], 1.0)

            # ---- projections + RoPE (q,k hd-major; v token-major + ones col)
            with (
                tc.tile_pool(name="pa", bufs=5, space="PSUM") as pa,
                tc.tile_pool(name="pb", bufs=2, space="PSUM") as pb,
                tc.tile_pool(name="ptr", bufs=1, space="PSUM") as ptr,
                tc.tile_pool(name="xk", bufs=6) as xkp,
                tc.tile_pool(name="rtmp", bufs=2) as rtmp,
            ):
                for ch in range(NCH):
                    b = ch // NJ
                    tcol = ch * CH               # global token col
                    kcol = CH * (ch % NJ)        # within-batch token col
                    kvp = pa.tile([P, CH], F32, tag="pacc")
                    q0p = pa.tile([P, CH], F32, tag="pacc")
                    q1p = pa.tile([P, CH], F32, tag="pacc")
                    for k in range(KT):
                        xk = xkp.tile([P, CH], BF16, tag="xk")
                        nc.sync.dma_start(
                            xk[:], xT_d[k * P:(k + 1) * P, tcol:tcol + CH])
                        st = (k == 0)
                        sp = (k == KT - 1)
                        nc.tensor.matmul(kvp[:], wkv_sb[:, k, :], xk[:],
                                         start=st, stop=sp)
                        nc.tensor.matmul(q0p[:], wq_sb[:, k, 0:P], xk[:],
                                         start=st, stop=sp)
                        nc.tensor.matmul(q1p[:], wq_sb[:, k, P:256], xk[:],
                                         start=st, stop=sp)
                    csl = cs_sb[:, kcol:kcol + CH]
                    snl = sn_sb[:, kcol:kcol + CH]
                    # q RoPE: rope(q) = q*cos + (P.T@q)*sin
                    for ht, qp in ((0, q0p), (1, q1p)):
                        qs = rtmp.tile([P, CH], BF16, tag="ropea")
                        if ht == 0:
                            nc.scalar.copy(qs[:], qp[:])
                        else:
                            nc.vector.tensor_copy(qs[:], qp[:])
                        qsw = pb.tile([P, CH], F32, tag="pswap")
                        nc.tensor.matmul(qsw[:], perm_sb[:], qs[:],
                                         start=True, stop=True)
                        dst = q_sb[:, ht, tcol:tcol + CH]
                        nc.gpsimd.tensor_mul(dst, qs[:], csl)
                        t2 = rtmp.tile([P, CH], BF16, tag="ropeb")
                        nc.vector.tensor_mul(t2[:], qsw[:], snl)
                        nc.vector.tensor_add(dst, dst, t2[:])
                    # k RoPE (rows 0:64 of kv psum), then duplicate to 64:128
                    ks = rtmp.tile([HD, CH], BF16, tag="ropek")
                    nc.scalar.copy(ks[:], kvp[0:HD, :])
                    ksw_full = pb.tile([P, CH], F32, tag="pswap")
                    ksw = ksw_full[0:HD, :]
                    nc.tensor.matmul(ksw, perm_sb[0:HD, 0:HD], ks[:],
                                     start=True, stop=True)
                    kdst = k_sb[0:HD, b, kcol:kcol + CH]
                    nc.vector.tensor_mul(kdst, ks[:], cs_sb[0:HD, kcol:kcol + CH])
                    t2 = rtmp.tile([HD, CH], BF16, tag="ropekb")
                    nc.vector.tensor_mul(t2[:], ksw, sn_sb[0:HD, kcol:kcol + CH])
                    nc.vector.tensor_add(kdst, kdst, t2[:])
                    nc.gpsimd.tensor_copy(k_sb[HD:P, b, kcol:kcol + CH], kdst)
                    # v: copy rows 64:128, transpose 128-tok tiles to token-major
                    vs = rtmp.tile([HD, CH], F32, tag="ropev")
                    nc.scalar.copy(vs[:], kvp[HD:P, :])
                    for tt in range(CH // P):
                        vtp = ptr.tile([P, HD], F32, tag="ptr")
                        nc.tensor.transpose(vtp[:], vs[:, tt * P:(tt + 1) * P],
                                            id64_sb[:])
                        nc.vector.tensor_copy(
                            v_sb[:, ch * (CH // P) + tt, 0:HD], vtp[:])

            # ---- attention + software-pipelined output projection
            # Out-proj matmuls of chunk N-1 are emitted INSIDE chunk N's
            # attention i-loop: the PE FIFO is in-order, and the exp
            # (ACT) rate-limits attV, so these fill the PE stalls that
            # otherwise trigger HAM downclocking.
            with (
                tc.tile_pool(name="po", bufs=2, space="PSUM") as po,
                tc.tile_pool(name="ps", bufs=2, space="PSUM") as ps,
                tc.tile_pool(name="pout", bufs=2, space="PSUM") as pout,
                tc.tile_pool(name="pp", bufs=8) as pp,
                tc.tile_pool(name="att", bufs=2) as att,
                tc.tile_pool(name="otp", bufs=6) as otp,
            ):
                pending = []  # deferred out-proj units: (og_c, tcol, tt, dc)

                def emit_outproj(og_p, tcol_p, tt, dc):
                    op = pout.tile([P, CH], F32, tag="pout")
                    for ft in range(2):
                        nc.tensor.matmul(
                            op[:],
                            og_p[:, ft, tt * P:(tt + 1) * P],
                            wo_sb[:, ft, dc * CH:(dc + 1) * CH],
                            start=(ft == 0), stop=(ft == 1))
                    ob = otp.tile([P, CH], BF16, tag="ob")
                    nc.vector.tensor_copy(ob[:], op[:])
                    nc.sync.dma_start(
                        out_d[tcol_p + tt * P:tcol_p + (tt + 1) * P,
                              dc * CH:(dc + 1) * CH],
                        ob[:])

                for b in range(B):
                    for j in range(NJ):
                        ch = b * NJ + j
                        tcol = ch * CH
                        imax = (CH // P) * j + (CH // P) - 1
                        og_c = att.tile([P, 2, CH], BF16, tag="og")
                        for hp in range(NHL // 2):
                            ot_acc = [po.tile([HD + 1, CH], F32, tag="po",
                                              name=f"po_{ch}_{h}")
                                      for h in (2 * hp, 2 * hp + 1)]
                            for i in range(imax + 1):
                                c0 = max(0, P * i - CH * j)
                                sp2 = ps.tile([P, 2, CH], F32, tag="ps")
                                for hi, h in enumerate((2 * hp, 2 * hp + 1)):
                                    hb = HD * (h % 2)
                                    ht = h // 2
                                    nc.tensor.matmul(
                                        sp2[:, hi, c0:CH],
                                        k_sb[hb:hb + HD, b, P * i:P * (i + 1)],
                                        q_sb[hb:hb + HD, ht, tcol + c0:tcol + CH],
                                        start=True, stop=True)
                                if P * i >= CH * j:  # diagonal: causal mask
                                    nc.vector.tensor_add(
                                        sp2[:, :, c0:c0 + P],
                                        sp2[:, :, c0:c0 + P],
                                        dmask_sb[:])
                                pt2 = pp.tile([P, 2, CH], BF16, tag="pt")
                                nc.scalar.activation(
                                    pt2[:, :, c0:CH], sp2[:, :, c0:CH],
                                    mybir.ActivationFunctionType.Exp,
                                    scale=0.125)
                                for hi in range(2):
                                    nc.tensor.matmul(
                                        ot_acc[hi][:, c0:CH],
                                        v_sb[:, b * NI + i, :],
                                        pt2[:, hi, c0:CH],
                                        start=(i == 0), stop=(i == imax),
                                        skip_group_check=True)
                                if pending:
                                    emit_outproj(*pending.pop(0))
                            for hi, h in enumerate((2 * hp, 2 * hp + 1)):
                                hb = HD * (h % 2)
                                ht = h // 2
                                # softmax denominator row -> SBUF partition 0
                                # (recip_approx misreads PSUM at partition
                                # offset 64), approx-reciprocal, broadcast
                                # across partitions on GpSimd
                                otr = otp.tile([1, CH], F32, tag="otr")
                                nc.vector.tensor_copy(
                                    otr[:], ot_acc[hi][HD:HD + 1, :])
                                dr = otp.tile([1, CH], F32, tag="dr")
                                nc.vector.reciprocal_approx_fast(
                                    dr[:], otr[:])
                                lr = otp.tile([HD, CH], F32, tag="lr")
                                nc.gpsimd.partition_broadcast(lr[:], dr[:])
                                nc.vector.tensor_mul(
                                    og_c[hb:hb + HD, ht, :],
                                    ot_acc[hi][0:HD, :], lr[:])
                            for _ in range(4):
                                if pending:
                                    emit_outproj(*pending.pop(0))
                        pending.extend(
                            (og_c, tcol, tt, dc)
                            for tt in range(CH // P) for dc in range(D // CH))
                while pending:
                    emit_outproj(*pending.pop(0))
    nc.compile()
    return nc


def host_prep(x, wq, wk, wv, wo, cos, sin, core, B=2, T=2048, D=2048):
    """Per-core input map. Core c owns KV head c and Q heads 4c..4c+3."""
    BT = B * T
    xT = np.ascontiguousarray(x.reshape(BT, D).T.astype(NPBF16))
    wqT = np.ascontiguousarray(wq[256 * core:256 * (core + 1)].T.astype(NPBF16))
    wkvT = np.ascontiguousarray(
        np.concatenate([wk[HD * core:HD * (core + 1)],
                        wv[HD * core:HD * (core + 1)]], axis=0).T.astype(NPBF16))
    woT = np.ascontiguousarray(wo[:, 256 * core:256 * (core + 1)].T.astype(NPBF16))
    idx = (np.arange(P) % HD) // 2
    cs = np.ascontiguousarray(cos[:T, idx].T.astype(NPBF16))
    sn = np.ascontiguousarray(sin[:T, idx].T.astype(NPBF16))
    perm = np.zeros((P, P), dtype=NPBF16)
    ii = np.arange(0, P, 2)
    perm[ii, ii + 1] = 1.0
    perm[ii + 1, ii] = -1.0
    dmask = np.where(np.arange(P)[:, None] <= np.arange(P)[None, :],
                     0.0, -1e10).astype(np.float32)
    return {
        "xT": xT, "wqT": wqT, "wkvT": wkvT, "woT": woT,
        "cs": cs, "sn": sn, "perm": perm,
        "id64": np.eye(HD, dtype=np.float32), "dmask": dmask,
    }


_CACHE = {}


def _get_program(B, T, D):
    key = (B, T, D)
    if key not in _CACHE:
        _CACHE[key] = build_program(B, T, D)
    return _CACHE[key]


def run_on_hw(x, wq, wk, wv, wo, cos, sin, B=2, T=2048, D=2048, trace=False, **kw):
    nc = _get_program(B, T, D)
    in_maps = [host_prep(x, wq, wk, wv, wo, cos, sin, c, B, T, D)
               for c in range(NCORES)]
    res = run_bass_kernel_spmd(nc, in_maps, list(range(NCORES)), trace=trace, **kw)
    parts = [np.asarray(r["out"], dtype=np.float32) for r in res.results]
    out = sum(parts).astype(np.float32).reshape(B, T, D)
    return out, res


def kernel(x, mask, wq, wk, wv, wo, cos, sin):
    x = np.asarray(x, dtype=np.float32)
    out, _ = run_on_hw(np.asarray(x, np.float32), np.asarray(wq, np.float32),
                       np.asarray(wk, np.float32), np.asarray(wv, np.float32),
                       np.asarray(wo, np.float32), np.asarray(cos, np.float32),
                       np.asarray(sin, np.float32))
    return out
